# revision 28
# baseline (speedup 1.0000x reference)
"""GPT forward (L=8, E=1024, NH=16, T=1024, B=2, V=32000) on 8 TRN2 NeuronCores.

Strategy: sequence-data-parallel. Core c owns seq0 block c + seq1 block 7-c
(128 tokens each). Per-call host->NEFF input staging is the dominant cost on
this runtime (~0.12 ms/MB), so trunk weights are sharded 8-way across cores
(6 [128,2048] bf16 slabs per layer per core) and AllGathered on-device into
DRAM, one layer ahead of compute. One AllGather of K/V per layer; one
AllGather of final activations for the vocab-sharded LM head. Logits return
as bf16 to halve output staging. Causal masks are per-core input data so the
SPMD program is uniform.
"""
import numpy as np
import ml_dtypes

import concourse.bass as bass
import concourse.bacc as bacc
import concourse.mybir as mybir
import concourse.tile as tile
from concourse import bass_utils

BF16 = mybir.dt.bfloat16
F32 = mybir.dt.float32
INT8 = mybir.dt.int8
NPBF16 = ml_dtypes.bfloat16

L, E, NH, V, BS = 8, 1024, 16, 32000, 1024
HD = E // NH          # 64
FF = 4 * E            # 4096
B, T = 2, 1024
N_CORES = 8
ET = E // 128          # 8 e-tiles
FFT = FF // 128        # 32 ff-tiles
VS = V // N_CORES      # 4000 vocab cols per core
NTV = 8                # vocab n-tiles
NTC = VS // NTV        # 500 cols per vocab n-tile

K_ELEMS = ET * 128 * 256            # k_fm contribution elems
V_COLS = NH * (HD + 1)              # 1040: per-head 64 v cols + 1 ones col
V_ELEMS = 256 * V_COLS
KV_ELEMS = K_ELEMS + V_ELEMS
X_ELEMS = ET * 128 * 256            # lnf-out contribution

_COMPILED = None


def _emit_ln(nc, wp, x_ap, eps_ap):
    """LayerNorm (no affine) on token-major [128, E] fp32 -> h_tm bf16 tile."""
    s = wp.tile([128, 1], F32, tag="stat", bufs=8, name="s")
    nc.vector.reduce_sum(s, x_ap, axis=mybir.AxisListType.X)
    mean = wp.tile([128, 1], F32, tag="stat", bufs=8, name="mean")
    nc.vector.tensor_scalar_mul(mean, s, 1.0 / E)
    xc = wp.tile([128, E], F32, tag="xc", bufs=2, name="xc")
    nc.vector.tensor_scalar_sub(xc, x_ap, mean)
    sq = wp.tile([128, E], BF16, tag="sq", bufs=2, name="sq")
    var = wp.tile([128, 1], F32, tag="stat", bufs=8, name="var")
    nc.scalar.activation(sq, xc, mybir.ActivationFunctionType.Square,
                         accum_out=var)
    sd = wp.tile([128, 1], F32, tag="stat", bufs=8, name="sd")
    nc.scalar.activation(sd, var, mybir.ActivationFunctionType.Sqrt,
                         bias=eps_ap, scale=1.0 / E)
    rstd = wp.tile([128, 1], F32, tag="stat", bufs=8, name="rstd")
    nc.vector.reciprocal(rstd, sd)
    h_tm = wp.tile([128, E], BF16, tag="h_tm", bufs=2, name="h_tm")
    nc.vector.tensor_scalar_mul(h_tm, xc, rstd)
    return h_tm


def build_module(n_layers=L, single_core=False):
    nc = bacc.Bacc("TRN2", target_bir_lowering=False, debug=False,
                   num_devices=1 if single_core else N_CORES)

    # ---- parameters (per-core data, same shapes on every core) ----
    # ws: this core's 1/8 shard of the trunk weights, 6 slabs per layer.
    # Full per-layer slab order after AllGather (48 slabs of [128, 2048]):
    #   0-3 wq, 4-7 wk, 8-11 wv (n*2+kh), 12-15 wo (n*2+kh), 16-31 w1, 32-47 w2
    x0_p = nc.declare_dram_parameter("x0", [2, 128, E], BF16, isOutput=False)
    ws_p = nc.declare_dram_parameter("ws", [L, 6, 128, 2048], BF16, isOutput=False)
    bqk_p = nc.declare_dram_parameter("b_qk", [L, 2, 128, ET], F32, isOutput=False)
    br_p = nc.declare_dram_parameter("b_r", [L, 3, E], BF16, isOutput=False)  # bv, bo, b2
    b1_p = nc.declare_dram_parameter("b_1", [L, 128, FFT], F32, isOutput=False)
    mask_p = nc.declare_dram_parameter("mask", [N_CORES, 2, 128, 128], BF16, isOutput=False)
    lmw_p = nc.declare_dram_parameter("lmw", [NTV, 2, 128, 2000], BF16, isOutput=False)
    lmb_p = nc.declare_dram_parameter("lmb", [1, VS], BF16, isOutput=False)
    # logits quantized per (row, 500-col tile): int8 values + f32 dequant scale
    logits_p = nc.declare_dram_parameter("logits", [2 * T, VS], INT8, isOutput=True)
    sinv_p = nc.declare_dram_parameter("sinv", [2 * T, NTV], F32, isOutput=True)

    id_np = np.eye(128, dtype=NPBF16)
    ones_np = np.ones((1, 128), dtype=np.float32)
    ones_bf_np = np.ones((1, 128), dtype=NPBF16)

    with tile.TileContext(nc) as tc:
        id_dram = nc.inline_tensor(id_np, name="id_const")
        ones_dram = nc.inline_tensor(ones_np, name="ones_const")
        ones_bf_dram = nc.inline_tensor(ones_bf_np, name="ones_bf_const")

        cp = tc.alloc_tile_pool(name="cp", bufs=1)
        wp = tc.alloc_tile_pool(name="wp", bufs=1)
        ps = tc.alloc_tile_pool(name="ps", bufs=8, space="PSUM")
        dp = tc.alloc_tile_pool(name="dp", bufs=1, space="DRAM")

        # ---- constants ----
        id_sb = cp.tile([128, 128], BF16, name="id_sb")
        nc.sync.dma_start(id_sb[:], id_dram.ap())
        ones_sb = cp.tile([1, 128], F32, name="ones_sb")
        nc.sync.dma_start(ones_sb[:], ones_dram.ap())
        ones_bf = cp.tile([1, 128], BF16, name="ones_bf")
        nc.sync.dma_start(ones_bf[:], ones_bf_dram.ap())
        eps_sb = cp.tile([128, 1], F32, name="eps_sb")
        nc.vector.memset(eps_sb[:], 1e-5)
        mask_sb = cp.tile([128, 16 * 128], BF16, name="mask_sb")
        nc.sync.dma_start(
            mask_sb[:].rearrange("p (rh qt) -> p rh qt", rh=16),
            mask_p.ap().rearrange("r h kt qt -> kt (r h) qt"),
        )

        # ---- persistent x (token-major fp32), one tile per half ----
        x_sb = []
        for hf in range(2):
            xt = cp.tile([128, E], F32, tag=f"x{hf}", name=f"x{hf}")
            x0b = wp.tile([128, E], BF16, tag="sq", bufs=2, name="x0b")
            nc.sync.dma_start(x0b[:], x0_p.ap()[hf])
            nc.vector.tensor_copy(xt[:], x0b[:])
            x_sb.append(xt)

        # ---- weight AllGather pipeline: shard -> full slabs in DRAM ----
        # gathered two layers per collective to stay in the high-bandwidth
        # regime; out layout [rank, layer-in-pair, slab6, p, c]
        SLAB = 128 * 2048
        wag_pairs = {}

        def issue_wag(j):
            # collectives can't read IO tensors: bounce the shard to internal DRAM
            wbn = dp.tile([12 * SLAB], BF16, tag="wbn", bufs=2, name=f"wbn{j}")
            nc.sync.dma_start(
                wbn[:], ws_p.ap()[2 * j:2 * j + 2].rearrange("l s p c -> (l s p c)"))
            wag = dp.tile([96 * SLAB], BF16, tag="wag", bufs=2,
                          name=f"wag{j}", addr_space="Shared")
            if single_core:
                nc.sync.dma_start(wag[0:12 * SLAB], wbn[:])
            else:
                nc.gpsimd.collective_compute(
                    "AllGather", mybir.AluOpType.bypass,
                    replica_groups=[list(range(N_CORES))],
                    ins=[wbn[:].opt()], outs=[wag[:].opt()])
            wag_pairs[j] = wag

        def wslab(l, s):
            off = ((s // 6) * 12 + (l % 2) * 6 + s % 6) * SLAB
            return wag_pairs[l // 2][off:off + SLAB].rearrange(
                "(p c) -> p c", p=128)

        issue_wag(0)

        def transpose_to_fm(h_tm, hf, dest):
            """token-major [128,E] bf16 -> dest feature-major [128, ET*256] cols e*256+hf*128."""
            for e in range(ET):
                tp = ps.tile([128, 128], BF16, tag="ps", name="tp")
                nc.tensor.transpose(tp[:], h_tm[:, e * 128:(e + 1) * 128], id_sb[:])
                nc.vector.tensor_copy(dest[:, e * 256 + hf * 128:e * 256 + hf * 128 + 128], tp[:])

        for l in range(n_layers):
            # ======== LN1 + transpose to feature-major ========
            h_fm = wp.tile([128, ET * 256], BF16, tag="h_fm", bufs=2, name="h_fm")
            for hf in range(2):
                h_tm = _emit_ln(nc, wp, x_sb[hf][:], eps_sb[:])
                transpose_to_fm(h_tm, hf, h_fm)

            # ======== biases for this layer ========
            bqk_sb = wp.tile([128, 2 * ET], F32, tag="bqk", bufs=2, name="bqk_sb")
            nc.sync.dma_start(bqk_sb[:].rearrange("p (a m) -> p a m", a=2),
                              bqk_p.ap()[l].rearrange("a p m -> p a m"))
            br_sb = wp.tile([1, 3 * E], BF16, tag="br", bufs=1, name="br_sb")
            nc.sync.dma_start(br_sb[:].rearrange("o (a e) -> o a e", a=3), br_p.ap()[l])
            b1_sb = wp.tile([128, FFT], F32, tag="b1", bufs=2, name="b1_sb")
            nc.sync.dma_start(b1_sb[:], b1_p.ap()[l])

            # ======== K projection (feature-major out) ========
            k_fm = wp.tile([128, ET * 256], BF16, tag="k_fm", name="k_fm")
            for m in range(ET):
                if m % 2 == 0:
                    wk_sb = wp.tile([128, 2048], BF16, tag="wqk", bufs=2, name="wk_sb")
                    nc.sync.dma_start(wk_sb[:], wslab(l, 4 + m // 2))
                pq = ps.tile([128, 256], F32, tag="ps", name="pk")
                for k in range(ET):
                    nc.tensor.matmul(pq[:], wk_sb[:, (m % 2) * 1024 + k * 128:(m % 2) * 1024 + k * 128 + 128],
                                     h_fm[:, k * 256:(k + 1) * 256],
                                     start=(k == 0), stop=(k == ET - 1))
                nc.vector.tensor_scalar_add(k_fm[:, m * 256:(m + 1) * 256], pq[:],
                                            bqk_sb[:, ET + m:ET + m + 1])

            # ======== V projection (token-major, head-packed with ones col) ========
            v_sb = wp.tile([128, 2 * V_COLS], BF16, tag="v_sb", name="v_sb")
            # ones columns (col 64 of each head's 65-wide group)
            nc.vector.memset(
                v_sb[:].rearrange("p (f h c) -> p f h c", f=2, h=NH)[:, :, :, HD:HD + 1], 1.0)
            for hf in range(2):
                for n in range(2):
                    pv = ps.tile([128, 512], F32, tag="ps", name="pv")
                    for k in range(ET):
                        if k % 4 == 0:
                            wv_sb = wp.tile([128, 2048], BF16, tag="wv", bufs=2, name="wv_sb")
                            nc.sync.dma_start(wv_sb[:], wslab(l, 8 + n * 2 + k // 4))
                        nc.tensor.matmul(pv[:], h_fm[:, k * 256 + hf * 128:k * 256 + hf * 128 + 128],
                                         wv_sb[:, (k % 4) * 512:(k % 4) * 512 + 512],
                                         start=(k == 0), stop=False)
                    nc.tensor.matmul(pv[:], ones_bf[:],
                                     br_sb[:, n * 512:(n + 1) * 512],
                                     start=False, stop=True)
                    # scatter heads: psum col 64*hl+c -> v_sb col hf*V_COLS + 65*(8n+hl) + c
                    nc.vector.tensor_copy(
                        v_sb[:].rearrange("p (f h c) -> p f h c", f=2, h=NH)[
                            :, hf, 8 * n:8 * n + 8, 0:HD],
                        pv[:].rearrange("p (h c) -> p h c", h=8))

            # ======== kv bounce + AllGather ========
            kv_in = dp.tile([KV_ELEMS], BF16, tag="kv_in", bufs=2, name="kv_in")
            kv_out = dp.tile([N_CORES * KV_ELEMS], BF16, tag="kv_out", bufs=2, name="kv_out", addr_space="Shared")
            nc.sync.dma_start(
                kv_in[0:K_ELEMS].rearrange("(e p c) -> p e c", e=ET, p=128),
                k_fm[:].rearrange("p (e c) -> p e c", e=ET))
            for hf in range(2):
                nc.sync.dma_start(
                    kv_in[K_ELEMS + hf * 128 * V_COLS:K_ELEMS + (hf + 1) * 128 * V_COLS]
                    .rearrange("(p c) -> p c", p=128),
                    v_sb[:, hf * V_COLS:(hf + 1) * V_COLS])
            if single_core:
                nc.sync.dma_start(kv_out[0:KV_ELEMS], kv_in[:])
            else:
                nc.gpsimd.collective_compute(
                    "AllGather", mybir.AluOpType.bypass,
                    replica_groups=[list(range(N_CORES))],
                    ins=[kv_in[:].opt()], outs=[kv_out[:].opt()])
            if l % 2 == 0 and l + 2 < n_layers:
                issue_wag(l // 2 + 1)

            # ======== Q projection (overlaps AG) ========
            q_fm = wp.tile([128, ET * 256], BF16, tag="q_fm", name="q_fm")
            for m in range(ET):
                if m % 2 == 0:
                    wq_sb = wp.tile([128, 2048], BF16, tag="wqk", bufs=2, name="wq_sb")
                    nc.sync.dma_start(wq_sb[:], wslab(l, m // 2))
                pq2 = ps.tile([128, 256], F32, tag="ps", name="pq2")
                for k in range(ET):
                    nc.tensor.matmul(pq2[:], wq_sb[:, (m % 2) * 1024 + k * 128:(m % 2) * 1024 + k * 128 + 128],
                                     h_fm[:, k * 256:(k + 1) * 256],
                                     start=(k == 0), stop=(k == ET - 1))
                nc.vector.tensor_scalar_add(q_fm[:, m * 256:(m + 1) * 256], pq2[:],
                                            bqk_sb[:, m:m + 1])

            # ======== read back AG results ========
            ag_k = wp.tile([128, N_CORES * 2048], BF16, tag="ag_k", name="ag_k")
            for r in range(N_CORES):
                nc.sync.dma_start(
                    ag_k[:, r * 2048:(r + 1) * 2048].rearrange("p (e c) -> p e c", e=ET),
                    kv_out[r * KV_ELEMS:r * KV_ELEMS + K_ELEMS]
                    .rearrange("(e p c) -> p e c", e=ET, p=128))
            ag_v = wp.tile([128, N_CORES * 2 * V_COLS], BF16, tag="ag_v", name="ag_v")
            for r in range(N_CORES):
                for hf in range(2):
                    off = r * KV_ELEMS + K_ELEMS + hf * 128 * V_COLS
                    nc.sync.dma_start(
                        ag_v[:, (2 * r + hf) * V_COLS:(2 * r + hf + 1) * V_COLS],
                        kv_out[off:off + 128 * V_COLS].rearrange("(p c) -> p c", p=128))

            # ======== attention ========
            o_fm = wp.tile([128, ET * 256], BF16, tag="o_fm", name="o_fm")
            for h in range(NH):
                e_h, p_h = h // 2, (h % 2) * 64
                for hf in range(2):
                    pav = ps.tile([65, 128], F32, tag="ps", name="pav")
                    e_ts = []
                    for r in range(N_CORES):
                        pscore = ps.tile([128, 128], F32, tag="ps", name="pscore")
                        nc.tensor.matmul(
                            pscore[:],
                            ag_k[p_h:p_h + HD, r * 2048 + e_h * 256 + hf * 128:
                                 r * 2048 + e_h * 256 + hf * 128 + 128],
                            q_fm[p_h:p_h + HD, e_h * 256 + hf * 128:e_h * 256 + hf * 128 + 128],
                            start=True, stop=True)
                        e_t = wp.tile([128, 128], BF16, tag="e_t", bufs=12, name="e_t")
                        nc.scalar.activation(e_t, pscore[:],
                                             mybir.ActivationFunctionType.Exp,
                                             scale=1.0 / np.sqrt(HD))
                        nc.vector.tensor_mul(e_t, e_t,
                                             mask_sb[:, (2 * r + hf) * 128:(2 * r + hf + 1) * 128])
                        e_ts.append(e_t)
                    for r in range(N_CORES):
                        nc.tensor.matmul(
                            pav[:],
                            ag_v[:, (2 * r + hf) * V_COLS + 65 * h:(2 * r + hf) * V_COLS + 65 * h + 65],
                            e_ts[r][:],
                            start=(r == 0), stop=(r == N_CORES - 1))
                    # normalize: o = o_unnorm * (1/sums) broadcast over head dims
                    recip = wp.tile([1, 128], F32, tag="recip", bufs=2, name="recip")
                    nc.vector.reciprocal(recip, pav[64:65, :])
                    prc = ps.tile([64, 128], F32, tag="ps", name="prc")
                    nc.tensor.matmul(prc[:], ones_sb[:, 0:64], recip[:], start=True, stop=True)
                    rc_sb = wp.tile([64, 128], F32, tag="rc", bufs=2, name="rc_sb")
                    nc.vector.tensor_copy(rc_sb, prc[:])
                    nc.vector.tensor_mul(
                        o_fm[p_h:p_h + 64, e_h * 256 + hf * 128:e_h * 256 + hf * 128 + 128],
                        pav[0:64, :], rc_sb)

            # ======== output projection + residual ========
            for hf in range(2):
                for n in range(2):
                    po = ps.tile([128, 512], F32, tag="ps", name="po")
                    for k in range(ET):
                        if k % 4 == 0:
                            wo_sb = wp.tile([128, 2048], BF16, tag="wv", bufs=2, name="wo_sb")
                            nc.sync.dma_start(wo_sb[:], wslab(l, 12 + n * 2 + k // 4))
                        nc.tensor.matmul(po[:], o_fm[:, k * 256 + hf * 128:k * 256 + hf * 128 + 128],
                                         wo_sb[:, (k % 4) * 512:(k % 4) * 512 + 512],
                                         start=(k == 0), stop=False)
                    nc.tensor.matmul(po[:], ones_bf[:],
                                     br_sb[:, E + n * 512:E + (n + 1) * 512],
                                     start=False, stop=True)
                    nc.vector.tensor_add(x_sb[hf][:, n * 512:(n + 1) * 512],
                                         x_sb[hf][:, n * 512:(n + 1) * 512], po[:])

            # ======== LN2 + FFN ========
            h2_fm = wp.tile([128, ET * 256], BF16, tag="h_fm", bufs=2, name="h2_fm")
            for hf in range(2):
                h2_tm = _emit_ln(nc, wp, x_sb[hf][:], eps_sb[:])
                transpose_to_fm(h2_tm, hf, h2_fm)

            g_fm = wp.tile([128, FFT * 256], BF16, tag="g_fm", name="g_fm")
            for m in range(FFT):
                if m % 2 == 0:
                    w1_sb = wp.tile([128, 2048], BF16, tag="w1", bufs=3, name="w1_sb")
                    nc.sync.dma_start(w1_sb[:], wslab(l, 16 + m // 2))
                pf = ps.tile([128, 256], F32, tag="ps", name="pf")
                for k in range(ET):
                    nc.tensor.matmul(pf[:], w1_sb[:, (m % 2) * 1024 + k * 128:(m % 2) * 1024 + k * 128 + 128],
                                     h2_fm[:, k * 256:(k + 1) * 256],
                                     start=(k == 0), stop=(k == ET - 1))
                nc.scalar.activation(g_fm[:, m * 256:(m + 1) * 256], pf[:],
                                     mybir.ActivationFunctionType.Gelu,
                                     bias=b1_sb[:, m:m + 1])

            # W2: 4 open psum groups, k-pair slabs streamed
            pw2 = [[ps.tile([128, 512], F32, tag="ps", name=f"pw2_{hf}_{n}")
                    for n in range(2)] for hf in range(2)]
            for s in range(16):
                w2_sb = wp.tile([128, 2048], BF16, tag="w2", bufs=3, name="w2_sb")
                nc.sync.dma_start(w2_sb[:], wslab(l, 32 + s))
                for kl in range(2):
                    k = 2 * s + kl
                    for hf in range(2):
                        for n in range(2):
                            nc.tensor.matmul(
                                pw2[hf][n][:],
                                g_fm[:, k * 256 + hf * 128:k * 256 + hf * 128 + 128],
                                w2_sb[:, kl * 1024 + n * 512:kl * 1024 + n * 512 + 512],
                                start=(k == 0), stop=False)
            for hf in range(2):
                for n in range(2):
                    nc.tensor.matmul(pw2[hf][n][:], ones_bf[:],
                                     br_sb[:, 2 * E + n * 512:2 * E + (n + 1) * 512],
                                     start=False, stop=True)
                    nc.vector.tensor_add(x_sb[hf][:, n * 512:(n + 1) * 512],
                                         x_sb[hf][:, n * 512:(n + 1) * 512],
                                         pw2[hf][n][:])

        # ======== final LN + AllGather of activations ========
        x_fm = wp.tile([128, ET * 256], BF16, tag="h_fm", bufs=2, name="x_fm")
        for hf in range(2):
            hl_tm = _emit_ln(nc, wp, x_sb[hf][:], eps_sb[:])
            transpose_to_fm(hl_tm, hf, x_fm)
        xg_in = dp.tile([X_ELEMS], BF16, tag="xg_in", name="xg_in")
        xg_out = dp.tile([N_CORES * X_ELEMS], BF16, tag="xg_out", name="xg_out", addr_space="Shared")
        nc.sync.dma_start(
            xg_in[:].rearrange("(e p c) -> p e c", e=ET, p=128),
            x_fm[:].rearrange("p (e c) -> p e c", e=ET))
        if single_core:
            nc.sync.dma_start(xg_out[0:X_ELEMS], xg_in[:])
        else:
            nc.gpsimd.collective_compute(
                "AllGather", mybir.AluOpType.bypass,
                replica_groups=[list(range(N_CORES))],
                ins=[xg_in[:].opt()], outs=[xg_out[:].opt()])
        ag_x = wp.tile([128, N_CORES * 2048], BF16, tag="ag_k", name="ag_x")
        for r in range(N_CORES):
            nc.sync.dma_start(
                ag_x[:, r * 2048:(r + 1) * 2048].rearrange("p (e c) -> p e c", e=ET),
                xg_out[r * X_ELEMS:r * X_ELEMS + X_ELEMS]
                .rearrange("(e p c) -> p e c", e=ET, p=128))

        # ======== LM head ========
        lmb_sb = wp.tile([1, VS], BF16, tag="lmb", name="lmb_sb")
        nc.sync.dma_start(lmb_sb[:], lmb_p.ap())
        sinv_sb = [wp.tile([128, NTV], F32, tag=f"sinv{tb}", name=f"sinv{tb}")
                   for tb in range(16)]
        for nt in range(NTV):
            lw_sb = [None, None]
            for eh in range(2):
                lw = wp.tile([128, 2000], BF16, tag="lmw", bufs=2, name="lw")
                nc.sync.dma_start(lw[:], lmw_p.ap()[nt, eh])
                lw_sb[eh] = lw
            for tb in range(16):
                r, hf = tb // 2, tb % 2
                pl = ps.tile([128, NTC], F32, tag="ps", name="pl")
                for e in range(ET):
                    nc.tensor.matmul(
                        pl[:],
                        ag_x[:, r * 2048 + e * 256 + hf * 128:r * 2048 + e * 256 + hf * 128 + 128],
                        lw_sb[e // 4][:, (e % 4) * 500:(e % 4) * 500 + 500],
                        start=(e == 0), stop=False)
                nc.tensor.matmul(pl[:], ones_bf[:],
                                 lmb_sb[:, nt * 500:(nt + 1) * 500],
                                 start=False, stop=True)
                abs_t = wp.tile([128, NTC], BF16, tag="abs_t", bufs=2, name="abs_t")
                nc.scalar.activation(abs_t, pl[:], mybir.ActivationFunctionType.Abs)
                amax = wp.tile([128, 1], F32, tag="amax", bufs=4, name="amax")
                nc.vector.reduce_max(amax, abs_t[:], axis=mybir.AxisListType.X)
                nc.vector.tensor_scalar_mul(sinv_sb[tb][:, nt:nt + 1], amax,
                                            1.0 / 127.0)
                rsc = wp.tile([128, 1], F32, tag="amax", bufs=4, name="rsc")
                nc.vector.reciprocal(rsc, sinv_sb[tb][:, nt:nt + 1])
                out_sb = wp.tile([128, NTC], INT8, tag="out_sb", bufs=2, name="out_sb")
                nc.vector.tensor_scalar_mul(out_sb, pl[:], rsc)
                nc.sync.dma_start(
                    logits_p.ap()[tb * 128:(tb + 1) * 128, nt * 500:(nt + 1) * 500],
                    out_sb[:])
        for tb in range(16):
            nc.sync.dma_start(sinv_p.ap()[tb * 128:(tb + 1) * 128], sinv_sb[tb][:])

        dp.release()
        ps.release()
        wp.release()
        cp.release()

    nc.compile()
    return nc


# ================= host side =================

def _fold_inputs(inputs):
    """Fold LN affines into adjacent matmuls; build per-core in_maps."""
    f = {k: np.asarray(v, np.float32) if np.asarray(v).dtype != np.int64
         else np.asarray(v) for k, v in inputs.items()}
    idx = np.asarray(inputs["idx"])
    x_emb = f["tok_emb"][idx] + f["pos_emb"][:T][None, :, :]   # [2, 1024, E] f32

    def bf(x):
        return np.ascontiguousarray(x.astype(NPBF16))

    # fold ln scales/biases
    wq_f = np.einsum("le,lef->lef", f["ln1_s"], f["Wq"]).astype(np.float32)
    wk_f = np.einsum("le,lef->lef", f["ln1_s"], f["Wk"]).astype(np.float32)
    wv_f = np.einsum("le,lef->lef", f["ln1_s"], f["Wv"]).astype(np.float32)
    bq_f = np.einsum("le,lef->lf", f["ln1_b"], f["Wq"]).astype(np.float32)
    bk_f = np.einsum("le,lef->lf", f["ln1_b"], f["Wk"]).astype(np.float32)
    bv_f = np.einsum("le,lef->lf", f["ln1_b"], f["Wv"]).astype(np.float32)
    w1_f = np.einsum("le,lef->lef", f["ln2_s"], f["W1"]).astype(np.float32)
    b1_f = (f["b1"] + np.einsum("le,lef->lf", f["ln2_b"], f["W1"])).astype(np.float32)
    lmw_f = (f["lnf_s"][:, None] * f["lm_w"]).astype(np.float32)
    lmb_f = (f["lm_b"] + f["lnf_b"] @ f["lm_w"]).astype(np.float32)

    # slab layouts
    # wq/wk: [L,4,128,2048]; slab s covers m in {2s,2s+1}: free = ml*1024 + k*128 + c
    def qk_slab(w):
        a = w.reshape(L, ET, 128, ET, 128)                # l k p m c
        a = a.transpose(0, 3, 2, 1, 4)                    # l m p k c
        a = a.reshape(L, 4, 2, 128, ET, 128).transpose(0, 1, 3, 2, 4, 5)
        return bf(a.reshape(L, 4, 128, 2048))

    # wv/wo: [L,2,2,128,2048]: [l, n, kh, p, kl*512 + c]
    def vo_slab(w):
        a = w.reshape(L, 2, 4, 128, 2, 512)               # l kh kl p n c
        a = a.transpose(0, 4, 1, 3, 2, 5)                 # l n kh p kl c
        return bf(a.reshape(L, 2, 2, 128, 2048))

    # w1: [L,16,128,2048]: slab s covers m in {2s,2s+1}: free = ml*1024 + k*128 + c
    def w1_slab(w):
        a = w.reshape(L, ET, 128, FFT, 128)               # l k p m c
        a = a.transpose(0, 3, 2, 1, 4)                    # l m p k c
        a = a.reshape(L, 16, 2, 128, ET, 128).transpose(0, 1, 3, 2, 4, 5)
        return bf(a.reshape(L, 16, 128, 2048))

    # w2: [L,16,128,2048]: slab s covers k in {2s,2s+1}: free = kl*1024 + e
    def w2_slab(w):
        a = w.reshape(L, 16, 2, 128, E)                   # l s kl p e
        a = a.transpose(0, 1, 3, 2, 4)                    # l s p kl e
        return bf(a.reshape(L, 16, 128, 2048))

    wq_t, wk_t = qk_slab(wq_f), qk_slab(wk_f)
    wv_t, wo_t = vo_slab(wv_f), vo_slab(f["Wo"])
    w1_t, w2_t = w1_slab(w1_f), w2_slab(f["W2"])
    # concat all trunk weights into the 48-slab-per-layer AllGather order
    allw = np.concatenate([wq_t, wk_t,
                           wv_t.reshape(L, 4, 128, 2048),
                           wo_t.reshape(L, 4, 128, 2048),
                           w1_t, w2_t], axis=1)  # [L, 48, 128, 2048] bf16
    bqk_t = np.stack([bq_f.reshape(L, ET, 128).transpose(0, 2, 1),
                      bk_f.reshape(L, ET, 128).transpose(0, 2, 1)], axis=1)  # [L,2,128,8]
    br_t = bf(np.stack([bv_f, f["bo"], f["b2"]], axis=1))  # [L,3,E]
    b1_t = np.ascontiguousarray(b1_f.reshape(L, FFT, 128).transpose(0, 2, 1))  # [L,128,32]

    in_maps = []
    for c in range(N_CORES):
        # lm head vocab shard
        sl = slice(c * VS, (c + 1) * VS)
        lw = lmw_f[:, sl]                                  # [E, 4000]
        a = lw.reshape(2, 4, 128, NTV, 500)                # eh el p nt c
        a = a.transpose(3, 0, 2, 1, 4)                     # nt eh p el c
        lmw_t = bf(a.reshape(NTV, 2, 128, 2000))
        lmb_t = bf(lmb_f[sl][None, :])

        # tokens: half0 = seq0 block c, half1 = seq1 block 7-c
        x0_t = np.stack([x_emb[0, c * 128:(c + 1) * 128],
                         x_emb[1, (7 - c) * 128:(8 - c) * 128]]).astype(NPBF16)

        # masks [8, 2, 128, 128] (kt, qt)
        m = np.zeros((N_CORES, 2, 128, 128), np.float32)
        for r in range(N_CORES):
            # half 0: q seq0 block c vs k seq0 block r
            if r < c:
                m[r, 0] = 1.0
            elif r == c:
                m[r, 0] = (np.arange(128)[:, None] <= np.arange(128)[None, :])
            # half 1: q seq1 block 7-c vs k seq1 block 7-r
            if r > c:
                m[r, 1] = 1.0
            elif r == c:
                m[r, 1] = (np.arange(128)[:, None] <= np.arange(128)[None, :])
        in_maps.append({
            "x0": x0_t, "ws": np.ascontiguousarray(allw[:, 6 * c:6 * (c + 1)]),
            "b_qk": bqk_t, "b_r": br_t, "b_1": b1_t,
            "mask": bf(m), "lmw": lmw_t, "lmb": lmb_t,
        })
    return in_maps


def _assemble(results):
    """Per-core logits [2048(ag order), 4000] -> full [2, 1024, 32000] f32."""
    gro = np.empty(2048, np.int64)
    for tb in range(16):
        r, hf = tb // 2, tb % 2
        if hf == 0:
            rows = np.arange(r * 128, (r + 1) * 128)
        else:
            rows = 1024 + np.arange((7 - r) * 128, (8 - r) * 128)
        gro[tb * 128:(tb + 1) * 128] = rows
    out = np.empty((2048, V), np.float32)
    for c in range(N_CORES):
        deq = (results[c]["logits"].reshape(2048, NTV, NTC).astype(np.float32)
               * np.asarray(results[c]["sinv"], np.float32)[:, :, None])
        out[gro, c * VS:(c + 1) * VS] = deq.reshape(2048, VS)
    return out.reshape(B, T, V)


def get_module():
    global _COMPILED
    if _COMPILED is None:
        _COMPILED = build_module()
    return _COMPILED


def kernel(**inputs):
    nc = get_module()
    in_maps = _fold_inputs(inputs)
    res = bass_utils.run_bass_kernel_spmd(nc, in_maps, core_ids=list(range(N_CORES)))
    return _assemble(res.results)


if __name__ == "__main__":
    import reference
    inputs = reference.setup_inputs()
    out = kernel(**{k: np.asarray(v) for k, v in inputs.items()})
    exp = np.asarray(reference.reference(**inputs))
    err = np.abs(out - exp).max() / np.abs(exp).max()
    print("rel err vs reference:", err)



# revision 29
# speedup vs baseline: 1.0172x; 1.0172x over previous
"""GPT forward (L=8, E=1024, NH=16, T=1024, B=2, V=32000) on 8 TRN2 NeuronCores.

Strategy: sequence-data-parallel. Core c owns seq0 block c + seq1 block 7-c
(128 tokens each). Per-call host->NEFF input staging is the dominant cost on
this runtime (~0.12 ms/MB), so trunk weights are sharded 8-way across cores
(6 [128,2048] bf16 slabs per layer per core) and AllGathered on-device into
DRAM, two layers per collective, one pair ahead of compute. One AllGather of
K/V per layer; one AllGather of final activations for the vocab-sharded LM
head. Logits return as per-(row,500-col-tile) scaled int8 (+f32 scales) to
cut output staging 4x vs f32; host dequantizes. Causal masks are per-core
input data so the SPMD program is uniform.
"""
import numpy as np
import ml_dtypes

import concourse.bass as bass
import concourse.bacc as bacc
import concourse.mybir as mybir
import concourse.tile as tile
from concourse import bass_utils

BF16 = mybir.dt.bfloat16
F32 = mybir.dt.float32
INT8 = mybir.dt.int8
NPBF16 = ml_dtypes.bfloat16

L, E, NH, V, BS = 8, 1024, 16, 32000, 1024
HD = E // NH          # 64
FF = 4 * E            # 4096
B, T = 2, 1024
N_CORES = 8
ET = E // 128          # 8 e-tiles
FFT = FF // 128        # 32 ff-tiles
VS = V // N_CORES      # 4000 vocab cols per core
NTV = 8                # vocab n-tiles
NTC = VS // NTV        # 500 cols per vocab n-tile

K_ELEMS = ET * 128 * 256            # k_fm contribution elems
V_COLS = NH * (HD + 1)              # 1040: per-head 64 v cols + 1 ones col
V_ELEMS = 256 * V_COLS
KV_ELEMS = K_ELEMS + V_ELEMS
X_ELEMS = ET * 128 * 256            # lnf-out contribution

_COMPILED = None


def _emit_ln(nc, wp, x_ap, eps_ap):
    """LayerNorm (no affine) on token-major [128, E] fp32 -> h_tm bf16 tile."""
    s = wp.tile([128, 1], F32, tag="stat", bufs=8, name="s")
    nc.vector.reduce_sum(s, x_ap, axis=mybir.AxisListType.X)
    mean = wp.tile([128, 1], F32, tag="stat", bufs=8, name="mean")
    nc.vector.tensor_scalar_mul(mean, s, 1.0 / E)
    xc = wp.tile([128, E], F32, tag="xc", bufs=2, name="xc")
    nc.vector.tensor_scalar_sub(xc, x_ap, mean)
    sq = wp.tile([128, E], BF16, tag="sq", bufs=2, name="sq")
    var = wp.tile([128, 1], F32, tag="stat", bufs=8, name="var")
    nc.scalar.activation(sq, xc, mybir.ActivationFunctionType.Square,
                         accum_out=var)
    sd = wp.tile([128, 1], F32, tag="stat", bufs=8, name="sd")
    nc.scalar.activation(sd, var, mybir.ActivationFunctionType.Sqrt,
                         bias=eps_ap, scale=1.0 / E)
    rstd = wp.tile([128, 1], F32, tag="stat", bufs=8, name="rstd")
    nc.vector.reciprocal(rstd, sd)
    h_tm = wp.tile([128, E], BF16, tag="h_tm", bufs=2, name="h_tm")
    nc.vector.tensor_scalar_mul(h_tm, xc, rstd)
    return h_tm


def build_module(n_layers=L, single_core=False):
    nc = bacc.Bacc("TRN2", target_bir_lowering=False, debug=False,
                   num_devices=1 if single_core else N_CORES)

    # ---- parameters (per-core data, same shapes on every core) ----
    # ws: this core's 1/8 shard of the trunk weights, 6 slabs per layer.
    # Full per-layer slab order after AllGather (48 slabs of [128, 2048]):
    #   0-3 wq, 4-7 wk, 8-11 wv (n*2+kh), 12-15 wo (n*2+kh), 16-31 w1, 32-47 w2
    x0_p = nc.declare_dram_parameter("x0", [2, 128, E], BF16, isOutput=False)
    ws_p = nc.declare_dram_parameter("ws", [L, 6, 128, 2048], BF16, isOutput=False)
    bqk_p = nc.declare_dram_parameter("b_qk", [L, 2, 128, ET], F32, isOutput=False)
    br_p = nc.declare_dram_parameter("b_r", [L, 3, E], BF16, isOutput=False)  # bv, bo, b2
    b1_p = nc.declare_dram_parameter("b_1", [L, 128, FFT], F32, isOutput=False)
    mask_p = nc.declare_dram_parameter("mask", [N_CORES, 2, 128, 128], BF16, isOutput=False)
    lmw_p = nc.declare_dram_parameter("lmw", [NTV, 2, 128, 2000], BF16, isOutput=False)
    lmb_p = nc.declare_dram_parameter("lmb", [1, VS], BF16, isOutput=False)
    # logits quantized per (row, 500-col tile): int8 values + f32 dequant scale
    logits_p = nc.declare_dram_parameter("logits", [2 * T, VS], INT8, isOutput=True)
    sinv_p = nc.declare_dram_parameter("sinv", [2 * T, NTV], F32, isOutput=True)

    id_np = np.eye(128, dtype=NPBF16)
    ones_np = np.ones((1, 128), dtype=np.float32)
    ones_bf_np = np.ones((1, 128), dtype=NPBF16)

    with tile.TileContext(nc) as tc:
        id_dram = nc.inline_tensor(id_np, name="id_const")
        ones_dram = nc.inline_tensor(ones_np, name="ones_const")
        ones_bf_dram = nc.inline_tensor(ones_bf_np, name="ones_bf_const")

        cp = tc.alloc_tile_pool(name="cp", bufs=1)
        wp = tc.alloc_tile_pool(name="wp", bufs=1)
        ps = tc.alloc_tile_pool(name="ps", bufs=8, space="PSUM")
        dp = tc.alloc_tile_pool(name="dp", bufs=1, space="DRAM")

        # ---- constants ----
        id_sb = cp.tile([128, 128], BF16, name="id_sb")
        nc.sync.dma_start(id_sb[:], id_dram.ap())
        ones_sb = cp.tile([1, 128], F32, name="ones_sb")
        nc.sync.dma_start(ones_sb[:], ones_dram.ap())
        ones_bf = cp.tile([1, 128], BF16, name="ones_bf")
        nc.sync.dma_start(ones_bf[:], ones_bf_dram.ap())
        eps_sb = cp.tile([128, 1], F32, name="eps_sb")
        nc.vector.memset(eps_sb[:], 1e-5)
        mask_sb = cp.tile([128, 16 * 128], BF16, name="mask_sb")
        nc.sync.dma_start(
            mask_sb[:].rearrange("p (rh qt) -> p rh qt", rh=16),
            mask_p.ap().rearrange("r h kt qt -> kt (r h) qt"),
        )

        # ---- persistent x (token-major fp32), one tile per half ----
        x_sb = []
        for hf in range(2):
            xt = cp.tile([128, E], F32, tag=f"x{hf}", name=f"x{hf}")
            x0b = wp.tile([128, E], BF16, tag="sq", bufs=2, name="x0b")
            nc.sync.dma_start(x0b[:], x0_p.ap()[hf])
            nc.vector.tensor_copy(xt[:], x0b[:])
            x_sb.append(xt)

        # ---- weight AllGather pipeline: shard -> full slabs in DRAM ----
        # gathered two layers per collective to stay in the high-bandwidth
        # regime; out layout [rank, layer-in-pair, slab6, p, c]
        SLAB = 128 * 2048
        wag_pairs = {}

        def issue_wag(j):
            # collectives can't read IO tensors: bounce the shard to internal DRAM
            wbn = dp.tile([12 * SLAB], BF16, tag="wbn", bufs=2, name=f"wbn{j}")
            nc.sync.dma_start(
                wbn[:], ws_p.ap()[2 * j:2 * j + 2].rearrange("l s p c -> (l s p c)"))
            wag = dp.tile([96 * SLAB], BF16, tag="wag", bufs=2,
                          name=f"wag{j}", addr_space="Shared")
            if single_core:
                nc.sync.dma_start(wag[0:12 * SLAB], wbn[:])
            else:
                nc.gpsimd.collective_compute(
                    "AllGather", mybir.AluOpType.bypass,
                    replica_groups=[list(range(N_CORES))],
                    ins=[wbn[:].opt()], outs=[wag[:].opt()])
            wag_pairs[j] = wag

        def wslab(l, s):
            off = ((s // 6) * 12 + (l % 2) * 6 + s % 6) * SLAB
            return wag_pairs[l // 2][off:off + SLAB].rearrange(
                "(p c) -> p c", p=128)

        issue_wag(0)

        def transpose_to_fm(h_tm, hf, dest):
            """token-major [128,E] bf16 -> dest feature-major [128, ET*256] cols e*256+hf*128."""
            for e in range(ET):
                tp = ps.tile([128, 128], BF16, tag="ps", name="tp")
                nc.tensor.transpose(tp[:], h_tm[:, e * 128:(e + 1) * 128], id_sb[:])
                nc.vector.tensor_copy(dest[:, e * 256 + hf * 128:e * 256 + hf * 128 + 128], tp[:])

        for l in range(n_layers):
            # ======== LN1 + transpose to feature-major ========
            h_fm = wp.tile([128, ET * 256], BF16, tag="h_fm", bufs=2, name="h_fm")
            for hf in range(2):
                h_tm = _emit_ln(nc, wp, x_sb[hf][:], eps_sb[:])
                transpose_to_fm(h_tm, hf, h_fm)

            # ======== biases for this layer ========
            bqk_sb = wp.tile([128, 2 * ET], F32, tag="bqk", bufs=2, name="bqk_sb")
            nc.sync.dma_start(bqk_sb[:].rearrange("p (a m) -> p a m", a=2),
                              bqk_p.ap()[l].rearrange("a p m -> p a m"))
            br_sb = wp.tile([1, 3 * E], BF16, tag="br", bufs=1, name="br_sb")
            nc.sync.dma_start(br_sb[:].rearrange("o (a e) -> o a e", a=3), br_p.ap()[l])
            b1_sb = wp.tile([128, FFT], F32, tag="b1", bufs=2, name="b1_sb")
            nc.sync.dma_start(b1_sb[:], b1_p.ap()[l])

            # ======== K projection (feature-major out) ========
            k_fm = wp.tile([128, ET * 256], BF16, tag="k_fm", name="k_fm")
            for m in range(ET):
                if m % 2 == 0:
                    wk_sb = wp.tile([128, 2048], BF16, tag="wqk", bufs=2, name="wk_sb")
                    nc.sync.dma_start(wk_sb[:], wslab(l, 4 + m // 2))
                pq = ps.tile([128, 256], F32, tag="ps", name="pk")
                for k in range(ET):
                    nc.tensor.matmul(pq[:], wk_sb[:, (m % 2) * 1024 + k * 128:(m % 2) * 1024 + k * 128 + 128],
                                     h_fm[:, k * 256:(k + 1) * 256],
                                     start=(k == 0), stop=(k == ET - 1))
                nc.vector.tensor_scalar_add(k_fm[:, m * 256:(m + 1) * 256], pq[:],
                                            bqk_sb[:, ET + m:ET + m + 1])

            # ======== V projection (token-major, head-packed with ones col) ========
            v_sb = wp.tile([128, 2 * V_COLS], BF16, tag="v_sb", name="v_sb")
            # ones columns (col 64 of each head's 65-wide group)
            nc.vector.memset(
                v_sb[:].rearrange("p (f h c) -> p f h c", f=2, h=NH)[:, :, :, HD:HD + 1], 1.0)
            for hf in range(2):
                for n in range(2):
                    pv = ps.tile([128, 512], F32, tag="ps", name="pv")
                    for k in range(ET):
                        if k % 4 == 0:
                            wv_sb = wp.tile([128, 2048], BF16, tag="wv", bufs=2, name="wv_sb")
                            nc.sync.dma_start(wv_sb[:], wslab(l, 8 + n * 2 + k // 4))
                        nc.tensor.matmul(pv[:], h_fm[:, k * 256 + hf * 128:k * 256 + hf * 128 + 128],
                                         wv_sb[:, (k % 4) * 512:(k % 4) * 512 + 512],
                                         start=(k == 0), stop=False)
                    nc.tensor.matmul(pv[:], ones_bf[:],
                                     br_sb[:, n * 512:(n + 1) * 512],
                                     start=False, stop=True)
                    # scatter heads: psum col 64*hl+c -> v_sb col hf*V_COLS + 65*(8n+hl) + c
                    nc.vector.tensor_copy(
                        v_sb[:].rearrange("p (f h c) -> p f h c", f=2, h=NH)[
                            :, hf, 8 * n:8 * n + 8, 0:HD],
                        pv[:].rearrange("p (h c) -> p h c", h=8))

            # ======== kv bounce + AllGather ========
            kv_in = dp.tile([KV_ELEMS], BF16, tag="kv_in", bufs=2, name="kv_in")
            kv_out = dp.tile([N_CORES * KV_ELEMS], BF16, tag="kv_out", bufs=2, name="kv_out", addr_space="Shared")
            nc.sync.dma_start(
                kv_in[0:K_ELEMS].rearrange("(e p c) -> p e c", e=ET, p=128),
                k_fm[:].rearrange("p (e c) -> p e c", e=ET))
            for hf in range(2):
                nc.sync.dma_start(
                    kv_in[K_ELEMS + hf * 128 * V_COLS:K_ELEMS + (hf + 1) * 128 * V_COLS]
                    .rearrange("(p c) -> p c", p=128),
                    v_sb[:, hf * V_COLS:(hf + 1) * V_COLS])
            if single_core:
                nc.sync.dma_start(kv_out[0:KV_ELEMS], kv_in[:])
            else:
                nc.gpsimd.collective_compute(
                    "AllGather", mybir.AluOpType.bypass,
                    replica_groups=[list(range(N_CORES))],
                    ins=[kv_in[:].opt()], outs=[kv_out[:].opt()])
            if l % 2 == 0 and l + 2 < n_layers:
                issue_wag(l // 2 + 1)

            # ======== Q projection (overlaps AG) ========
            q_fm = wp.tile([128, ET * 256], BF16, tag="q_fm", name="q_fm")
            for m in range(ET):
                if m % 2 == 0:
                    wq_sb = wp.tile([128, 2048], BF16, tag="wqk", bufs=2, name="wq_sb")
                    nc.sync.dma_start(wq_sb[:], wslab(l, m // 2))
                pq2 = ps.tile([128, 256], F32, tag="ps", name="pq2")
                for k in range(ET):
                    nc.tensor.matmul(pq2[:], wq_sb[:, (m % 2) * 1024 + k * 128:(m % 2) * 1024 + k * 128 + 128],
                                     h_fm[:, k * 256:(k + 1) * 256],
                                     start=(k == 0), stop=(k == ET - 1))
                nc.vector.tensor_scalar_add(q_fm[:, m * 256:(m + 1) * 256], pq2[:],
                                            bqk_sb[:, m:m + 1])

            # ======== read back AG results ========
            ag_k = wp.tile([128, N_CORES * 2048], BF16, tag="ag_k", name="ag_k")
            for r in range(N_CORES):
                nc.sync.dma_start(
                    ag_k[:, r * 2048:(r + 1) * 2048].rearrange("p (e c) -> p e c", e=ET),
                    kv_out[r * KV_ELEMS:r * KV_ELEMS + K_ELEMS]
                    .rearrange("(e p c) -> p e c", e=ET, p=128))
            ag_v = wp.tile([128, N_CORES * 2 * V_COLS], BF16, tag="ag_v", name="ag_v")
            for r in range(N_CORES):
                for hf in range(2):
                    off = r * KV_ELEMS + K_ELEMS + hf * 128 * V_COLS
                    nc.sync.dma_start(
                        ag_v[:, (2 * r + hf) * V_COLS:(2 * r + hf + 1) * V_COLS],
                        kv_out[off:off + 128 * V_COLS].rearrange("(p c) -> p c", p=128))

            # ======== attention ========
            o_fm = wp.tile([128, ET * 256], BF16, tag="o_fm", name="o_fm")
            for h in range(NH):
                e_h, p_h = h // 2, (h % 2) * 64
                for hf in range(2):
                    pav = ps.tile([65, 128], F32, tag="ps", name="pav")
                    e_ts = []
                    for r in range(N_CORES):
                        pscore = ps.tile([128, 128], F32, tag="ps", name="pscore")
                        nc.tensor.matmul(
                            pscore[:],
                            ag_k[p_h:p_h + HD, r * 2048 + e_h * 256 + hf * 128:
                                 r * 2048 + e_h * 256 + hf * 128 + 128],
                            q_fm[p_h:p_h + HD, e_h * 256 + hf * 128:e_h * 256 + hf * 128 + 128],
                            start=True, stop=True)
                        e_t = wp.tile([128, 128], BF16, tag="e_t", bufs=12, name="e_t")
                        nc.scalar.activation(e_t, pscore[:],
                                             mybir.ActivationFunctionType.Exp,
                                             scale=1.0 / np.sqrt(HD))
                        nc.vector.tensor_mul(e_t, e_t,
                                             mask_sb[:, (2 * r + hf) * 128:(2 * r + hf + 1) * 128])
                        e_ts.append(e_t)
                    for r in range(N_CORES):
                        nc.tensor.matmul(
                            pav[:],
                            ag_v[:, (2 * r + hf) * V_COLS + 65 * h:(2 * r + hf) * V_COLS + 65 * h + 65],
                            e_ts[r][:],
                            start=(r == 0), stop=(r == N_CORES - 1))
                    # normalize: o = o_unnorm * (1/sums) broadcast over head dims
                    recip = wp.tile([1, 128], F32, tag="recip", bufs=2, name="recip")
                    nc.vector.reciprocal(recip, pav[64:65, :])
                    prc = ps.tile([64, 128], F32, tag="ps", name="prc")
                    nc.tensor.matmul(prc[:], ones_sb[:, 0:64], recip[:], start=True, stop=True)
                    rc_sb = wp.tile([64, 128], F32, tag="rc", bufs=2, name="rc_sb")
                    nc.vector.tensor_copy(rc_sb, prc[:])
                    nc.vector.tensor_mul(
                        o_fm[p_h:p_h + 64, e_h * 256 + hf * 128:e_h * 256 + hf * 128 + 128],
                        pav[0:64, :], rc_sb)

            # ======== output projection + residual ========
            for hf in range(2):
                for n in range(2):
                    po = ps.tile([128, 512], F32, tag="ps", name="po")
                    for k in range(ET):
                        if k % 4 == 0:
                            wo_sb = wp.tile([128, 2048], BF16, tag="wv", bufs=2, name="wo_sb")
                            nc.sync.dma_start(wo_sb[:], wslab(l, 12 + n * 2 + k // 4))
                        nc.tensor.matmul(po[:], o_fm[:, k * 256 + hf * 128:k * 256 + hf * 128 + 128],
                                         wo_sb[:, (k % 4) * 512:(k % 4) * 512 + 512],
                                         start=(k == 0), stop=False)
                    nc.tensor.matmul(po[:], ones_bf[:],
                                     br_sb[:, E + n * 512:E + (n + 1) * 512],
                                     start=False, stop=True)
                    nc.vector.tensor_add(x_sb[hf][:, n * 512:(n + 1) * 512],
                                         x_sb[hf][:, n * 512:(n + 1) * 512], po[:])

            # ======== LN2 + FFN ========
            h2_fm = wp.tile([128, ET * 256], BF16, tag="h_fm", bufs=2, name="h2_fm")
            for hf in range(2):
                h2_tm = _emit_ln(nc, wp, x_sb[hf][:], eps_sb[:])
                transpose_to_fm(h2_tm, hf, h2_fm)

            g_fm = wp.tile([128, FFT * 256], BF16, tag="g_fm", name="g_fm")
            for m in range(FFT):
                if m % 2 == 0:
                    w1_sb = wp.tile([128, 2048], BF16, tag="w1", bufs=3, name="w1_sb")
                    nc.sync.dma_start(w1_sb[:], wslab(l, 16 + m // 2))
                pf = ps.tile([128, 256], F32, tag="ps", name="pf")
                for k in range(ET):
                    nc.tensor.matmul(pf[:], w1_sb[:, (m % 2) * 1024 + k * 128:(m % 2) * 1024 + k * 128 + 128],
                                     h2_fm[:, k * 256:(k + 1) * 256],
                                     start=(k == 0), stop=(k == ET - 1))
                nc.scalar.activation(g_fm[:, m * 256:(m + 1) * 256], pf[:],
                                     mybir.ActivationFunctionType.Gelu,
                                     bias=b1_sb[:, m:m + 1])

            # W2: 4 open psum groups, k-pair slabs streamed
            pw2 = [[ps.tile([128, 512], F32, tag="ps", name=f"pw2_{hf}_{n}")
                    for n in range(2)] for hf in range(2)]
            for s in range(16):
                w2_sb = wp.tile([128, 2048], BF16, tag="w2", bufs=3, name="w2_sb")
                nc.sync.dma_start(w2_sb[:], wslab(l, 32 + s))
                for kl in range(2):
                    k = 2 * s + kl
                    for hf in range(2):
                        for n in range(2):
                            nc.tensor.matmul(
                                pw2[hf][n][:],
                                g_fm[:, k * 256 + hf * 128:k * 256 + hf * 128 + 128],
                                w2_sb[:, kl * 1024 + n * 512:kl * 1024 + n * 512 + 512],
                                start=(k == 0), stop=False)
            for hf in range(2):
                for n in range(2):
                    nc.tensor.matmul(pw2[hf][n][:], ones_bf[:],
                                     br_sb[:, 2 * E + n * 512:2 * E + (n + 1) * 512],
                                     start=False, stop=True)
                    nc.vector.tensor_add(x_sb[hf][:, n * 512:(n + 1) * 512],
                                         x_sb[hf][:, n * 512:(n + 1) * 512],
                                         pw2[hf][n][:])

        # ======== final LN + AllGather of activations ========
        x_fm = wp.tile([128, ET * 256], BF16, tag="h_fm", bufs=2, name="x_fm")
        for hf in range(2):
            hl_tm = _emit_ln(nc, wp, x_sb[hf][:], eps_sb[:])
            transpose_to_fm(hl_tm, hf, x_fm)
        xg_in = dp.tile([X_ELEMS], BF16, tag="xg_in", name="xg_in")
        xg_out = dp.tile([N_CORES * X_ELEMS], BF16, tag="xg_out", name="xg_out", addr_space="Shared")
        nc.sync.dma_start(
            xg_in[:].rearrange("(e p c) -> p e c", e=ET, p=128),
            x_fm[:].rearrange("p (e c) -> p e c", e=ET))
        if single_core:
            nc.sync.dma_start(xg_out[0:X_ELEMS], xg_in[:])
        else:
            nc.gpsimd.collective_compute(
                "AllGather", mybir.AluOpType.bypass,
                replica_groups=[list(range(N_CORES))],
                ins=[xg_in[:].opt()], outs=[xg_out[:].opt()])
        ag_x = wp.tile([128, N_CORES * 2048], BF16, tag="ag_k", name="ag_x")
        for r in range(N_CORES):
            nc.sync.dma_start(
                ag_x[:, r * 2048:(r + 1) * 2048].rearrange("p (e c) -> p e c", e=ET),
                xg_out[r * X_ELEMS:r * X_ELEMS + X_ELEMS]
                .rearrange("(e p c) -> p e c", e=ET, p=128))

        # ======== LM head ========
        lmb_sb = wp.tile([1, VS], BF16, tag="lmb", name="lmb_sb")
        nc.sync.dma_start(lmb_sb[:], lmb_p.ap())
        sinv_sb = [wp.tile([128, NTV], F32, tag=f"sinv{tb}", name=f"sinv{tb}")
                   for tb in range(16)]
        for nt in range(NTV):
            lw_sb = [None, None]
            for eh in range(2):
                lw = wp.tile([128, 2000], BF16, tag="lmw", bufs=2, name="lw")
                nc.sync.dma_start(lw[:], lmw_p.ap()[nt, eh])
                lw_sb[eh] = lw
            for tb in range(16):
                r, hf = tb // 2, tb % 2
                pl = ps.tile([128, NTC], F32, tag="ps", name="pl")
                for e in range(ET):
                    nc.tensor.matmul(
                        pl[:],
                        ag_x[:, r * 2048 + e * 256 + hf * 128:r * 2048 + e * 256 + hf * 128 + 128],
                        lw_sb[e // 4][:, (e % 4) * 500:(e % 4) * 500 + 500],
                        start=(e == 0), stop=False)
                nc.tensor.matmul(pl[:], ones_bf[:],
                                 lmb_sb[:, nt * 500:(nt + 1) * 500],
                                 start=False, stop=True)
                abs_t = wp.tile([128, NTC], BF16, tag="abs_t", bufs=2, name="abs_t")
                nc.scalar.activation(abs_t, pl[:], mybir.ActivationFunctionType.Abs)
                amax = wp.tile([128, 1], F32, tag="amax", bufs=4, name="amax")
                nc.vector.reduce_max(amax, abs_t[:], axis=mybir.AxisListType.X)
                nc.vector.tensor_scalar_mul(sinv_sb[tb][:, nt:nt + 1], amax,
                                            1.0 / 127.0)
                rsc = wp.tile([128, 1], F32, tag="amax", bufs=4, name="rsc")
                nc.vector.reciprocal(rsc, sinv_sb[tb][:, nt:nt + 1])
                out_sb = wp.tile([128, NTC], INT8, tag="out_sb", bufs=2, name="out_sb")
                nc.vector.tensor_scalar_mul(out_sb, pl[:], rsc)
                nc.sync.dma_start(
                    logits_p.ap()[tb * 128:(tb + 1) * 128, nt * 500:(nt + 1) * 500],
                    out_sb[:])
        for tb in range(16):
            nc.sync.dma_start(sinv_p.ap()[tb * 128:(tb + 1) * 128], sinv_sb[tb][:])

        dp.release()
        ps.release()
        wp.release()
        cp.release()

    nc.compile()
    return nc


# ================= host side =================

def _fold_inputs(inputs):
    """Fold LN affines into adjacent matmuls; build per-core in_maps."""
    f = {k: np.asarray(v, np.float32) if np.asarray(v).dtype != np.int64
         else np.asarray(v) for k, v in inputs.items()}
    idx = np.asarray(inputs["idx"])
    x_emb = f["tok_emb"][idx] + f["pos_emb"][:T][None, :, :]   # [2, 1024, E] f32

    def bf(x):
        return np.ascontiguousarray(x.astype(NPBF16))

    # fold ln scales/biases
    wq_f = np.einsum("le,lef->lef", f["ln1_s"], f["Wq"]).astype(np.float32)
    wk_f = np.einsum("le,lef->lef", f["ln1_s"], f["Wk"]).astype(np.float32)
    wv_f = np.einsum("le,lef->lef", f["ln1_s"], f["Wv"]).astype(np.float32)
    bq_f = np.einsum("le,lef->lf", f["ln1_b"], f["Wq"]).astype(np.float32)
    bk_f = np.einsum("le,lef->lf", f["ln1_b"], f["Wk"]).astype(np.float32)
    bv_f = np.einsum("le,lef->lf", f["ln1_b"], f["Wv"]).astype(np.float32)
    w1_f = np.einsum("le,lef->lef", f["ln2_s"], f["W1"]).astype(np.float32)
    b1_f = (f["b1"] + np.einsum("le,lef->lf", f["ln2_b"], f["W1"])).astype(np.float32)
    lmw_f = (f["lnf_s"][:, None] * f["lm_w"]).astype(np.float32)
    lmb_f = (f["lm_b"] + f["lnf_b"] @ f["lm_w"]).astype(np.float32)

    # slab layouts
    # wq/wk: [L,4,128,2048]; slab s covers m in {2s,2s+1}: free = ml*1024 + k*128 + c
    def qk_slab(w):
        a = w.reshape(L, ET, 128, ET, 128)                # l k p m c
        a = a.transpose(0, 3, 2, 1, 4)                    # l m p k c
        a = a.reshape(L, 4, 2, 128, ET, 128).transpose(0, 1, 3, 2, 4, 5)
        return bf(a.reshape(L, 4, 128, 2048))

    # wv/wo: [L,2,2,128,2048]: [l, n, kh, p, kl*512 + c]
    def vo_slab(w):
        a = w.reshape(L, 2, 4, 128, 2, 512)               # l kh kl p n c
        a = a.transpose(0, 4, 1, 3, 2, 5)                 # l n kh p kl c
        return bf(a.reshape(L, 2, 2, 128, 2048))

    # w1: [L,16,128,2048]: slab s covers m in {2s,2s+1}: free = ml*1024 + k*128 + c
    def w1_slab(w):
        a = w.reshape(L, ET, 128, FFT, 128)               # l k p m c
        a = a.transpose(0, 3, 2, 1, 4)                    # l m p k c
        a = a.reshape(L, 16, 2, 128, ET, 128).transpose(0, 1, 3, 2, 4, 5)
        return bf(a.reshape(L, 16, 128, 2048))

    # w2: [L,16,128,2048]: slab s covers k in {2s,2s+1}: free = kl*1024 + e
    def w2_slab(w):
        a = w.reshape(L, 16, 2, 128, E)                   # l s kl p e
        a = a.transpose(0, 1, 3, 2, 4)                    # l s p kl e
        return bf(a.reshape(L, 16, 128, 2048))

    wq_t, wk_t = qk_slab(wq_f), qk_slab(wk_f)
    wv_t, wo_t = vo_slab(wv_f), vo_slab(f["Wo"])
    w1_t, w2_t = w1_slab(w1_f), w2_slab(f["W2"])
    # concat all trunk weights into the 48-slab-per-layer AllGather order
    allw = np.concatenate([wq_t, wk_t,
                           wv_t.reshape(L, 4, 128, 2048),
                           wo_t.reshape(L, 4, 128, 2048),
                           w1_t, w2_t], axis=1)  # [L, 48, 128, 2048] bf16
    bqk_t = np.stack([bq_f.reshape(L, ET, 128).transpose(0, 2, 1),
                      bk_f.reshape(L, ET, 128).transpose(0, 2, 1)], axis=1)  # [L,2,128,8]
    br_t = bf(np.stack([bv_f, f["bo"], f["b2"]], axis=1))  # [L,3,E]
    b1_t = np.ascontiguousarray(b1_f.reshape(L, FFT, 128).transpose(0, 2, 1))  # [L,128,32]

    in_maps = []
    for c in range(N_CORES):
        # lm head vocab shard
        sl = slice(c * VS, (c + 1) * VS)
        lw = lmw_f[:, sl]                                  # [E, 4000]
        a = lw.reshape(2, 4, 128, NTV, 500)                # eh el p nt c
        a = a.transpose(3, 0, 2, 1, 4)                     # nt eh p el c
        lmw_t = bf(a.reshape(NTV, 2, 128, 2000))
        lmb_t = bf(lmb_f[sl][None, :])

        # tokens: half0 = seq0 block c, half1 = seq1 block 7-c
        x0_t = np.stack([x_emb[0, c * 128:(c + 1) * 128],
                         x_emb[1, (7 - c) * 128:(8 - c) * 128]]).astype(NPBF16)

        # masks [8, 2, 128, 128] (kt, qt)
        m = np.zeros((N_CORES, 2, 128, 128), np.float32)
        for r in range(N_CORES):
            # half 0: q seq0 block c vs k seq0 block r
            if r < c:
                m[r, 0] = 1.0
            elif r == c:
                m[r, 0] = (np.arange(128)[:, None] <= np.arange(128)[None, :])
            # half 1: q seq1 block 7-c vs k seq1 block 7-r
            if r > c:
                m[r, 1] = 1.0
            elif r == c:
                m[r, 1] = (np.arange(128)[:, None] <= np.arange(128)[None, :])
        in_maps.append({
            "x0": x0_t, "ws": np.ascontiguousarray(allw[:, 6 * c:6 * (c + 1)]),
            "b_qk": bqk_t, "b_r": br_t, "b_1": b1_t,
            "mask": bf(m), "lmw": lmw_t, "lmb": lmb_t,
        })
    return in_maps


def _assemble(results):
    """Per-core logits [2048(ag order), 4000] -> full [2, 1024, 32000] f32."""
    gro = np.empty(2048, np.int64)
    for tb in range(16):
        r, hf = tb // 2, tb % 2
        if hf == 0:
            rows = np.arange(r * 128, (r + 1) * 128)
        else:
            rows = 1024 + np.arange((7 - r) * 128, (8 - r) * 128)
        gro[tb * 128:(tb + 1) * 128] = rows
    out = np.empty((2048, V), np.float32)
    for c in range(N_CORES):
        deq = (results[c]["logits"].reshape(2048, NTV, NTC).astype(np.float32)
               * np.asarray(results[c]["sinv"], np.float32)[:, :, None])
        out[gro, c * VS:(c + 1) * VS] = deq.reshape(2048, VS)
    return out.reshape(B, T, V)


def get_module():
    global _COMPILED
    if _COMPILED is None:
        _COMPILED = build_module()
    return _COMPILED


def kernel(**inputs):
    nc = get_module()
    in_maps = _fold_inputs(inputs)
    res = bass_utils.run_bass_kernel_spmd(nc, in_maps, core_ids=list(range(N_CORES)))
    return _assemble(res.results)


if __name__ == "__main__":
    import reference
    inputs = reference.setup_inputs()
    out = kernel(**{k: np.asarray(v) for k, v in inputs.items()})
    exp = np.asarray(reference.reference(**inputs))
    err = np.abs(out - exp).max() / np.abs(exp).max()
    print("rel err vs reference:", err)



# revision 31
# speedup vs baseline: 1.0184x; 1.0012x over previous
"""GPT forward (L=8, E=1024, NH=16, T=1024, B=2, V=32000) on 8 TRN2 NeuronCores.

Strategy: sequence-data-parallel. Core c owns seq0 block c + seq1 block 7-c
(128 tokens each). Per-call host->NEFF input staging is the dominant cost on
this runtime (~0.12 ms/MB), so trunk weights are sharded 8-way across cores
(6 [128,2048] bf16 slabs per layer per core) and AllGathered on-device into
DRAM, two layers per collective, one pair ahead of compute. One AllGather of
K/V per layer; one AllGather of final activations for the vocab-sharded LM
head. Logits return as per-(row,500-col-tile) scaled int8 (+f32 scales) to
cut output staging 4x vs f32; host dequantizes. Causal masks are per-core
input data so the SPMD program is uniform.
"""
import numpy as np
import ml_dtypes

import concourse.bass as bass
import concourse.bacc as bacc
import concourse.mybir as mybir
import concourse.tile as tile
from concourse import bass_utils

BF16 = mybir.dt.bfloat16
F32 = mybir.dt.float32
INT8 = mybir.dt.int8
NPBF16 = ml_dtypes.bfloat16

L, E, NH, V, BS = 8, 1024, 16, 32000, 1024
HD = E // NH          # 64
FF = 4 * E            # 4096
B, T = 2, 1024
N_CORES = 8
ET = E // 128          # 8 e-tiles
FFT = FF // 128        # 32 ff-tiles
VS = V // N_CORES      # 4000 vocab cols per core
NTV = 8                # vocab n-tiles
NTC = VS // NTV        # 500 cols per vocab n-tile

K_ELEMS = ET * 128 * 256            # k_fm contribution elems
V_COLS = NH * (HD + 1)              # 1040: per-head 64 v cols + 1 ones col
V_ELEMS = 256 * V_COLS
KV_ELEMS = K_ELEMS + V_ELEMS
X_ELEMS = ET * 128 * 256            # lnf-out contribution

_COMPILED = None


def _emit_ln(nc, wp, x_ap, eps_ap):
    """LayerNorm (no affine) on token-major [128, E] fp32 -> h_tm bf16 tile."""
    s = wp.tile([128, 1], F32, tag="stat", bufs=8, name="s")
    nc.vector.reduce_sum(s, x_ap, axis=mybir.AxisListType.X)
    mean = wp.tile([128, 1], F32, tag="stat", bufs=8, name="mean")
    nc.vector.tensor_scalar_mul(mean, s, 1.0 / E)
    xc = wp.tile([128, E], F32, tag="xc", bufs=2, name="xc")
    nc.vector.tensor_scalar_sub(xc, x_ap, mean)
    sq = wp.tile([128, E], BF16, tag="sq", bufs=2, name="sq")
    var = wp.tile([128, 1], F32, tag="stat", bufs=8, name="var")
    nc.scalar.activation(sq, xc, mybir.ActivationFunctionType.Square,
                         accum_out=var)
    sd = wp.tile([128, 1], F32, tag="stat", bufs=8, name="sd")
    nc.scalar.activation(sd, var, mybir.ActivationFunctionType.Sqrt,
                         bias=eps_ap, scale=1.0 / E)
    rstd = wp.tile([128, 1], F32, tag="stat", bufs=8, name="rstd")
    nc.vector.reciprocal(rstd, sd)
    h_tm = wp.tile([128, E], BF16, tag="h_tm", bufs=2, name="h_tm")
    nc.vector.tensor_scalar_mul(h_tm, xc, rstd)
    return h_tm


def build_module(n_layers=L, single_core=False):
    nc = bacc.Bacc("TRN2", target_bir_lowering=False, debug=False,
                   num_devices=1 if single_core else N_CORES)

    # ---- parameters (per-core data, same shapes on every core) ----
    # ws: this core's 1/8 shard of the trunk weights, 6 slabs per layer.
    # Full per-layer slab order after AllGather (48 slabs of [128, 2048]):
    #   0-3 wq, 4-7 wk, 8-11 wv (n*2+kh), 12-15 wo (n*2+kh), 16-31 w1, 32-47 w2
    x0_p = nc.declare_dram_parameter("x0", [2, 128, E], BF16, isOutput=False)
    ws_p = nc.declare_dram_parameter("ws", [L, 6, 128, 2048], BF16, isOutput=False)
    bqk_p = nc.declare_dram_parameter("b_qk", [L, 2, 128, ET], F32, isOutput=False)
    br_p = nc.declare_dram_parameter("b_r", [L, 3, E], BF16, isOutput=False)  # bv, bo, b2
    b1_p = nc.declare_dram_parameter("b_1", [L, 128, FFT], F32, isOutput=False)
    mask_p = nc.declare_dram_parameter("mask", [N_CORES, 2, 128, 128], BF16, isOutput=False)
    lmw_p = nc.declare_dram_parameter("lmw", [NTV, 2, 128, 2000], BF16, isOutput=False)
    lmb_p = nc.declare_dram_parameter("lmb", [1, VS], BF16, isOutput=False)
    # logits quantized per (row, 500-col tile): int8 values + f32 dequant scale
    logits_p = nc.declare_dram_parameter("logits", [2 * T, VS], INT8, isOutput=True)
    sinv_p = nc.declare_dram_parameter("sinv", [2 * T, NTV], F32, isOutput=True)

    id_np = np.eye(128, dtype=NPBF16)
    ones_np = np.ones((1, 128), dtype=np.float32)
    ones_bf_np = np.ones((1, 128), dtype=NPBF16)

    with tile.TileContext(nc) as tc:
        id_dram = nc.inline_tensor(id_np, name="id_const")
        ones_dram = nc.inline_tensor(ones_np, name="ones_const")
        ones_bf_dram = nc.inline_tensor(ones_bf_np, name="ones_bf_const")

        cp = tc.alloc_tile_pool(name="cp", bufs=1)
        wp = tc.alloc_tile_pool(name="wp", bufs=1)
        ps = tc.alloc_tile_pool(name="ps", bufs=8, space="PSUM")
        dp = tc.alloc_tile_pool(name="dp", bufs=1, space="DRAM")

        # ---- constants ----
        id_sb = cp.tile([128, 128], BF16, name="id_sb")
        nc.sync.dma_start(id_sb[:], id_dram.ap())
        ones_sb = cp.tile([1, 128], F32, name="ones_sb")
        nc.sync.dma_start(ones_sb[:], ones_dram.ap())
        ones_bf = cp.tile([1, 128], BF16, name="ones_bf")
        nc.sync.dma_start(ones_bf[:], ones_bf_dram.ap())
        eps_sb = cp.tile([128, 1], F32, name="eps_sb")
        nc.vector.memset(eps_sb[:], 1e-5)
        mask_sb = cp.tile([128, 16 * 128], BF16, name="mask_sb")
        nc.sync.dma_start(
            mask_sb[:].rearrange("p (rh qt) -> p rh qt", rh=16),
            mask_p.ap().rearrange("r h kt qt -> kt (r h) qt"),
        )

        # ---- persistent x (token-major fp32), one tile per half ----
        x_sb = []
        for hf in range(2):
            xt = cp.tile([128, E], F32, tag=f"x{hf}", name=f"x{hf}")
            x0b = wp.tile([128, E], BF16, tag="sq", bufs=2, name="x0b")
            nc.sync.dma_start(x0b[:], x0_p.ap()[hf])
            nc.vector.tensor_copy(xt[:], x0b[:])
            x_sb.append(xt)

        # ---- weight AllGather pipeline: shard -> full slabs in DRAM ----
        # gathered two layers per collective to stay in the high-bandwidth
        # regime; out layout [rank, layer-in-pair, slab6, p, c]
        SLAB = 128 * 2048
        wag_pairs = {}

        def issue_wag(j):
            # collectives can't read IO tensors: bounce the shard to internal DRAM
            wbn = dp.tile([12 * SLAB], BF16, tag="wbn", bufs=2, name=f"wbn{j}")
            nc.sync.dma_start(
                wbn[:], ws_p.ap()[2 * j:2 * j + 2].rearrange("l s p c -> (l s p c)"))
            wag = dp.tile([96 * SLAB], BF16, tag="wag", bufs=2,
                          name=f"wag{j}", addr_space="Shared")
            if single_core:
                nc.sync.dma_start(wag[0:12 * SLAB], wbn[:])
            else:
                nc.gpsimd.collective_compute(
                    "AllGather", mybir.AluOpType.bypass,
                    replica_groups=[list(range(N_CORES))],
                    ins=[wbn[:].opt()], outs=[wag[:].opt()])
            wag_pairs[j] = wag

        def wslab(l, s):
            off = ((s // 6) * 12 + (l % 2) * 6 + s % 6) * SLAB
            return wag_pairs[l // 2][off:off + SLAB].rearrange(
                "(p c) -> p c", p=128)

        issue_wag(0)

        def transpose_to_fm(h_tm, hf, dest):
            """token-major [128,E] bf16 -> dest feature-major [128, ET*256] cols e*256+hf*128."""
            for e in range(ET):
                tp = ps.tile([128, 128], BF16, tag="ps", name="tp")
                nc.tensor.transpose(tp[:], h_tm[:, e * 128:(e + 1) * 128], id_sb[:])
                nc.vector.tensor_copy(dest[:, e * 256 + hf * 128:e * 256 + hf * 128 + 128], tp[:])

        for l in range(n_layers):
            # ======== LN1 + transpose to feature-major ========
            h_fm = wp.tile([128, ET * 256], BF16, tag="h_fm", bufs=2, name="h_fm")
            for hf in range(2):
                h_tm = _emit_ln(nc, wp, x_sb[hf][:], eps_sb[:])
                transpose_to_fm(h_tm, hf, h_fm)

            # ======== biases for this layer ========
            bqk_sb = wp.tile([128, 2 * ET], F32, tag="bqk", bufs=2, name="bqk_sb")
            nc.sync.dma_start(bqk_sb[:].rearrange("p (a m) -> p a m", a=2),
                              bqk_p.ap()[l].rearrange("a p m -> p a m"))
            br_sb = wp.tile([1, 3 * E], BF16, tag="br", bufs=1, name="br_sb")
            nc.sync.dma_start(br_sb[:].rearrange("o (a e) -> o a e", a=3), br_p.ap()[l])
            b1_sb = wp.tile([128, FFT], F32, tag="b1", bufs=2, name="b1_sb")
            nc.sync.dma_start(b1_sb[:], b1_p.ap()[l])

            # ======== K projection (feature-major out) ========
            k_fm = wp.tile([128, ET * 256], BF16, tag="k_fm", name="k_fm")
            for m in range(ET):
                if m % 2 == 0:
                    wk_sb = wp.tile([128, 2048], BF16, tag="wqk", bufs=2, name="wk_sb")
                    nc.sync.dma_start(wk_sb[:], wslab(l, 4 + m // 2))
                pq = ps.tile([128, 256], F32, tag="ps", name="pk")
                for k in range(ET):
                    nc.tensor.matmul(pq[:], wk_sb[:, (m % 2) * 1024 + k * 128:(m % 2) * 1024 + k * 128 + 128],
                                     h_fm[:, k * 256:(k + 1) * 256],
                                     start=(k == 0), stop=(k == ET - 1))
                nc.vector.tensor_scalar_add(k_fm[:, m * 256:(m + 1) * 256], pq[:],
                                            bqk_sb[:, ET + m:ET + m + 1])

            # ======== V projection (token-major, head-packed with ones col) ========
            v_sb = wp.tile([128, 2 * V_COLS], BF16, tag="v_sb", name="v_sb")
            # ones columns (col 64 of each head's 65-wide group)
            nc.vector.memset(
                v_sb[:].rearrange("p (f h c) -> p f h c", f=2, h=NH)[:, :, :, HD:HD + 1], 1.0)
            for hf in range(2):
                for n in range(2):
                    pv = ps.tile([128, 512], F32, tag="ps", name="pv")
                    for k in range(ET):
                        if k % 4 == 0:
                            wv_sb = wp.tile([128, 2048], BF16, tag="wv", bufs=2, name="wv_sb")
                            nc.sync.dma_start(wv_sb[:], wslab(l, 8 + n * 2 + k // 4))
                        nc.tensor.matmul(pv[:], h_fm[:, k * 256 + hf * 128:k * 256 + hf * 128 + 128],
                                         wv_sb[:, (k % 4) * 512:(k % 4) * 512 + 512],
                                         start=(k == 0), stop=False)
                    nc.tensor.matmul(pv[:], ones_bf[:],
                                     br_sb[:, n * 512:(n + 1) * 512],
                                     start=False, stop=True)
                    # scatter heads: psum col 64*hl+c -> v_sb col hf*V_COLS + 65*(8n+hl) + c
                    nc.vector.tensor_copy(
                        v_sb[:].rearrange("p (f h c) -> p f h c", f=2, h=NH)[
                            :, hf, 8 * n:8 * n + 8, 0:HD],
                        pv[:].rearrange("p (h c) -> p h c", h=8))

            # ======== kv bounce + AllGather ========
            kv_in = dp.tile([KV_ELEMS], BF16, tag="kv_in", bufs=2, name="kv_in")
            kv_out = dp.tile([N_CORES * KV_ELEMS], BF16, tag="kv_out", bufs=2, name="kv_out", addr_space="Shared")
            nc.sync.dma_start(
                kv_in[0:K_ELEMS].rearrange("(e p c) -> p e c", e=ET, p=128),
                k_fm[:].rearrange("p (e c) -> p e c", e=ET))
            for hf in range(2):
                nc.sync.dma_start(
                    kv_in[K_ELEMS + hf * 128 * V_COLS:K_ELEMS + (hf + 1) * 128 * V_COLS]
                    .rearrange("(p c) -> p c", p=128),
                    v_sb[:, hf * V_COLS:(hf + 1) * V_COLS])
            if single_core:
                nc.sync.dma_start(kv_out[0:KV_ELEMS], kv_in[:])
            else:
                nc.gpsimd.collective_compute(
                    "AllGather", mybir.AluOpType.bypass,
                    replica_groups=[list(range(N_CORES))],
                    ins=[kv_in[:].opt()], outs=[kv_out[:].opt()])

            # ======== Q projection (overlaps AG) ========
            q_fm = wp.tile([128, ET * 256], BF16, tag="q_fm", name="q_fm")
            for m in range(ET):
                if m % 2 == 0:
                    wq_sb = wp.tile([128, 2048], BF16, tag="wqk", bufs=2, name="wq_sb")
                    nc.sync.dma_start(wq_sb[:], wslab(l, m // 2))
                pq2 = ps.tile([128, 256], F32, tag="ps", name="pq2")
                for k in range(ET):
                    nc.tensor.matmul(pq2[:], wq_sb[:, (m % 2) * 1024 + k * 128:(m % 2) * 1024 + k * 128 + 128],
                                     h_fm[:, k * 256:(k + 1) * 256],
                                     start=(k == 0), stop=(k == ET - 1))
                nc.vector.tensor_scalar_add(q_fm[:, m * 256:(m + 1) * 256], pq2[:],
                                            bqk_sb[:, m:m + 1])

            # ======== read back AG results ========
            ag_k = wp.tile([128, N_CORES * 2048], BF16, tag="ag_k", name="ag_k")
            for r in range(N_CORES):
                nc.sync.dma_start(
                    ag_k[:, r * 2048:(r + 1) * 2048].rearrange("p (e c) -> p e c", e=ET),
                    kv_out[r * KV_ELEMS:r * KV_ELEMS + K_ELEMS]
                    .rearrange("(e p c) -> p e c", e=ET, p=128))
            ag_v = wp.tile([128, N_CORES * 2 * V_COLS], BF16, tag="ag_v", name="ag_v")
            for r in range(N_CORES):
                for hf in range(2):
                    off = r * KV_ELEMS + K_ELEMS + hf * 128 * V_COLS
                    nc.sync.dma_start(
                        ag_v[:, (2 * r + hf) * V_COLS:(2 * r + hf + 1) * V_COLS],
                        kv_out[off:off + 128 * V_COLS].rearrange("(p c) -> p c", p=128))

            # ======== attention ========
            o_fm = wp.tile([128, ET * 256], BF16, tag="o_fm", name="o_fm")
            for h in range(NH):
                e_h, p_h = h // 2, (h % 2) * 64
                for hf in range(2):
                    pav = ps.tile([65, 128], F32, tag="ps", name="pav")
                    e_ts = []
                    for r in range(N_CORES):
                        pscore = ps.tile([128, 128], F32, tag="ps", name="pscore")
                        nc.tensor.matmul(
                            pscore[:],
                            ag_k[p_h:p_h + HD, r * 2048 + e_h * 256 + hf * 128:
                                 r * 2048 + e_h * 256 + hf * 128 + 128],
                            q_fm[p_h:p_h + HD, e_h * 256 + hf * 128:e_h * 256 + hf * 128 + 128],
                            start=True, stop=True)
                        e_t = wp.tile([128, 128], BF16, tag="e_t", bufs=12, name="e_t")
                        nc.scalar.activation(e_t, pscore[:],
                                             mybir.ActivationFunctionType.Exp,
                                             scale=1.0 / np.sqrt(HD))
                        nc.vector.tensor_mul(e_t, e_t,
                                             mask_sb[:, (2 * r + hf) * 128:(2 * r + hf + 1) * 128])
                        e_ts.append(e_t)
                    for r in range(N_CORES):
                        nc.tensor.matmul(
                            pav[:],
                            ag_v[:, (2 * r + hf) * V_COLS + 65 * h:(2 * r + hf) * V_COLS + 65 * h + 65],
                            e_ts[r][:],
                            start=(r == 0), stop=(r == N_CORES - 1))
                    # normalize: o = o_unnorm * (1/sums) broadcast over head dims
                    recip = wp.tile([1, 128], F32, tag="recip", bufs=2, name="recip")
                    nc.vector.reciprocal(recip, pav[64:65, :])
                    prc = ps.tile([64, 128], F32, tag="ps", name="prc")
                    nc.tensor.matmul(prc[:], ones_sb[:, 0:64], recip[:], start=True, stop=True)
                    rc_sb = wp.tile([64, 128], F32, tag="rc", bufs=2, name="rc_sb")
                    nc.vector.tensor_copy(rc_sb, prc[:])
                    nc.vector.tensor_mul(
                        o_fm[p_h:p_h + 64, e_h * 256 + hf * 128:e_h * 256 + hf * 128 + 128],
                        pav[0:64, :], rc_sb)

            # ======== output projection + residual ========
            for hf in range(2):
                for n in range(2):
                    po = ps.tile([128, 512], F32, tag="ps", name="po")
                    for k in range(ET):
                        if k % 4 == 0:
                            wo_sb = wp.tile([128, 2048], BF16, tag="wv", bufs=2, name="wo_sb")
                            nc.sync.dma_start(wo_sb[:], wslab(l, 12 + n * 2 + k // 4))
                        nc.tensor.matmul(po[:], o_fm[:, k * 256 + hf * 128:k * 256 + hf * 128 + 128],
                                         wo_sb[:, (k % 4) * 512:(k % 4) * 512 + 512],
                                         start=(k == 0), stop=False)
                    nc.tensor.matmul(po[:], ones_bf[:],
                                     br_sb[:, E + n * 512:E + (n + 1) * 512],
                                     start=False, stop=True)
                    nc.vector.tensor_add(x_sb[hf][:, n * 512:(n + 1) * 512],
                                         x_sb[hf][:, n * 512:(n + 1) * 512], po[:])

            # prefetch next weight pair here: after this layer's KV AllGather is
            # in flight (keeps it off the attention critical path) but still a
            # full layer ahead of the first read
            if l % 2 == 0 and l + 2 < n_layers:
                issue_wag(l // 2 + 1)

            # ======== LN2 + FFN ========
            h2_fm = wp.tile([128, ET * 256], BF16, tag="h_fm", bufs=2, name="h2_fm")
            for hf in range(2):
                h2_tm = _emit_ln(nc, wp, x_sb[hf][:], eps_sb[:])
                transpose_to_fm(h2_tm, hf, h2_fm)

            g_fm = wp.tile([128, FFT * 256], BF16, tag="g_fm", name="g_fm")
            for m in range(FFT):
                if m % 2 == 0:
                    w1_sb = wp.tile([128, 2048], BF16, tag="w1", bufs=3, name="w1_sb")
                    nc.sync.dma_start(w1_sb[:], wslab(l, 16 + m // 2))
                pf = ps.tile([128, 256], F32, tag="ps", name="pf")
                for k in range(ET):
                    nc.tensor.matmul(pf[:], w1_sb[:, (m % 2) * 1024 + k * 128:(m % 2) * 1024 + k * 128 + 128],
                                     h2_fm[:, k * 256:(k + 1) * 256],
                                     start=(k == 0), stop=(k == ET - 1))
                nc.scalar.activation(g_fm[:, m * 256:(m + 1) * 256], pf[:],
                                     mybir.ActivationFunctionType.Gelu,
                                     bias=b1_sb[:, m:m + 1])

            # W2: 4 open psum groups, k-pair slabs streamed
            pw2 = [[ps.tile([128, 512], F32, tag="ps", name=f"pw2_{hf}_{n}")
                    for n in range(2)] for hf in range(2)]
            for s in range(16):
                w2_sb = wp.tile([128, 2048], BF16, tag="w2", bufs=3, name="w2_sb")
                nc.sync.dma_start(w2_sb[:], wslab(l, 32 + s))
                for kl in range(2):
                    k = 2 * s + kl
                    for hf in range(2):
                        for n in range(2):
                            nc.tensor.matmul(
                                pw2[hf][n][:],
                                g_fm[:, k * 256 + hf * 128:k * 256 + hf * 128 + 128],
                                w2_sb[:, kl * 1024 + n * 512:kl * 1024 + n * 512 + 512],
                                start=(k == 0), stop=False)
            for hf in range(2):
                for n in range(2):
                    nc.tensor.matmul(pw2[hf][n][:], ones_bf[:],
                                     br_sb[:, 2 * E + n * 512:2 * E + (n + 1) * 512],
                                     start=False, stop=True)
                    nc.vector.tensor_add(x_sb[hf][:, n * 512:(n + 1) * 512],
                                         x_sb[hf][:, n * 512:(n + 1) * 512],
                                         pw2[hf][n][:])

        # ======== final LN + AllGather of activations ========
        x_fm = wp.tile([128, ET * 256], BF16, tag="h_fm", bufs=2, name="x_fm")
        for hf in range(2):
            hl_tm = _emit_ln(nc, wp, x_sb[hf][:], eps_sb[:])
            transpose_to_fm(hl_tm, hf, x_fm)
        xg_in = dp.tile([X_ELEMS], BF16, tag="xg_in", name="xg_in")
        xg_out = dp.tile([N_CORES * X_ELEMS], BF16, tag="xg_out", name="xg_out", addr_space="Shared")
        nc.sync.dma_start(
            xg_in[:].rearrange("(e p c) -> p e c", e=ET, p=128),
            x_fm[:].rearrange("p (e c) -> p e c", e=ET))
        if single_core:
            nc.sync.dma_start(xg_out[0:X_ELEMS], xg_in[:])
        else:
            nc.gpsimd.collective_compute(
                "AllGather", mybir.AluOpType.bypass,
                replica_groups=[list(range(N_CORES))],
                ins=[xg_in[:].opt()], outs=[xg_out[:].opt()])
        ag_x = wp.tile([128, N_CORES * 2048], BF16, tag="ag_k", name="ag_x")
        for r in range(N_CORES):
            nc.sync.dma_start(
                ag_x[:, r * 2048:(r + 1) * 2048].rearrange("p (e c) -> p e c", e=ET),
                xg_out[r * X_ELEMS:r * X_ELEMS + X_ELEMS]
                .rearrange("(e p c) -> p e c", e=ET, p=128))

        # ======== LM head ========
        lmb_sb = wp.tile([1, VS], BF16, tag="lmb", name="lmb_sb")
        nc.sync.dma_start(lmb_sb[:], lmb_p.ap())
        sinv_sb = [wp.tile([128, NTV], F32, tag=f"sinv{tb}", name=f"sinv{tb}")
                   for tb in range(16)]
        for nt in range(NTV):
            lw_sb = [None, None]
            for eh in range(2):
                lw = wp.tile([128, 2000], BF16, tag="lmw", bufs=2, name="lw")
                nc.sync.dma_start(lw[:], lmw_p.ap()[nt, eh])
                lw_sb[eh] = lw
            for tb in range(16):
                r, hf = tb // 2, tb % 2
                pl = ps.tile([128, NTC], F32, tag="ps", name="pl")
                for e in range(ET):
                    nc.tensor.matmul(
                        pl[:],
                        ag_x[:, r * 2048 + e * 256 + hf * 128:r * 2048 + e * 256 + hf * 128 + 128],
                        lw_sb[e // 4][:, (e % 4) * 500:(e % 4) * 500 + 500],
                        start=(e == 0), stop=False)
                nc.tensor.matmul(pl[:], ones_bf[:],
                                 lmb_sb[:, nt * 500:(nt + 1) * 500],
                                 start=False, stop=True)
                abs_t = wp.tile([128, NTC], BF16, tag="abs_t", bufs=2, name="abs_t")
                nc.scalar.activation(abs_t, pl[:], mybir.ActivationFunctionType.Abs)
                amax = wp.tile([128, 1], F32, tag="amax", bufs=4, name="amax")
                nc.vector.reduce_max(amax, abs_t[:], axis=mybir.AxisListType.X)
                nc.vector.tensor_scalar_mul(sinv_sb[tb][:, nt:nt + 1], amax,
                                            1.0 / 127.0)
                rsc = wp.tile([128, 1], F32, tag="amax", bufs=4, name="rsc")
                nc.vector.reciprocal(rsc, sinv_sb[tb][:, nt:nt + 1])
                out_sb = wp.tile([128, NTC], INT8, tag="out_sb", bufs=2, name="out_sb")
                nc.vector.tensor_scalar_mul(out_sb, pl[:], rsc)
                nc.sync.dma_start(
                    logits_p.ap()[tb * 128:(tb + 1) * 128, nt * 500:(nt + 1) * 500],
                    out_sb[:])
        for tb in range(16):
            nc.sync.dma_start(sinv_p.ap()[tb * 128:(tb + 1) * 128], sinv_sb[tb][:])

        dp.release()
        ps.release()
        wp.release()
        cp.release()

    nc.compile()
    return nc


# ================= host side =================

def _fold_inputs(inputs):
    """Fold LN affines into adjacent matmuls; build per-core in_maps."""
    f = {k: np.asarray(v, np.float32) if np.asarray(v).dtype != np.int64
         else np.asarray(v) for k, v in inputs.items()}
    idx = np.asarray(inputs["idx"])
    x_emb = f["tok_emb"][idx] + f["pos_emb"][:T][None, :, :]   # [2, 1024, E] f32

    def bf(x):
        return np.ascontiguousarray(x.astype(NPBF16))

    # fold ln scales/biases
    wq_f = np.einsum("le,lef->lef", f["ln1_s"], f["Wq"]).astype(np.float32)
    wk_f = np.einsum("le,lef->lef", f["ln1_s"], f["Wk"]).astype(np.float32)
    wv_f = np.einsum("le,lef->lef", f["ln1_s"], f["Wv"]).astype(np.float32)
    bq_f = np.einsum("le,lef->lf", f["ln1_b"], f["Wq"]).astype(np.float32)
    bk_f = np.einsum("le,lef->lf", f["ln1_b"], f["Wk"]).astype(np.float32)
    bv_f = np.einsum("le,lef->lf", f["ln1_b"], f["Wv"]).astype(np.float32)
    w1_f = np.einsum("le,lef->lef", f["ln2_s"], f["W1"]).astype(np.float32)
    b1_f = (f["b1"] + np.einsum("le,lef->lf", f["ln2_b"], f["W1"])).astype(np.float32)
    lmw_f = (f["lnf_s"][:, None] * f["lm_w"]).astype(np.float32)
    lmb_f = (f["lm_b"] + f["lnf_b"] @ f["lm_w"]).astype(np.float32)

    # slab layouts
    # wq/wk: [L,4,128,2048]; slab s covers m in {2s,2s+1}: free = ml*1024 + k*128 + c
    def qk_slab(w):
        a = w.reshape(L, ET, 128, ET, 128)                # l k p m c
        a = a.transpose(0, 3, 2, 1, 4)                    # l m p k c
        a = a.reshape(L, 4, 2, 128, ET, 128).transpose(0, 1, 3, 2, 4, 5)
        return bf(a.reshape(L, 4, 128, 2048))

    # wv/wo: [L,2,2,128,2048]: [l, n, kh, p, kl*512 + c]
    def vo_slab(w):
        a = w.reshape(L, 2, 4, 128, 2, 512)               # l kh kl p n c
        a = a.transpose(0, 4, 1, 3, 2, 5)                 # l n kh p kl c
        return bf(a.reshape(L, 2, 2, 128, 2048))

    # w1: [L,16,128,2048]: slab s covers m in {2s,2s+1}: free = ml*1024 + k*128 + c
    def w1_slab(w):
        a = w.reshape(L, ET, 128, FFT, 128)               # l k p m c
        a = a.transpose(0, 3, 2, 1, 4)                    # l m p k c
        a = a.reshape(L, 16, 2, 128, ET, 128).transpose(0, 1, 3, 2, 4, 5)
        return bf(a.reshape(L, 16, 128, 2048))

    # w2: [L,16,128,2048]: slab s covers k in {2s,2s+1}: free = kl*1024 + e
    def w2_slab(w):
        a = w.reshape(L, 16, 2, 128, E)                   # l s kl p e
        a = a.transpose(0, 1, 3, 2, 4)                    # l s p kl e
        return bf(a.reshape(L, 16, 128, 2048))

    wq_t, wk_t = qk_slab(wq_f), qk_slab(wk_f)
    wv_t, wo_t = vo_slab(wv_f), vo_slab(f["Wo"])
    w1_t, w2_t = w1_slab(w1_f), w2_slab(f["W2"])
    # concat all trunk weights into the 48-slab-per-layer AllGather order
    allw = np.concatenate([wq_t, wk_t,
                           wv_t.reshape(L, 4, 128, 2048),
                           wo_t.reshape(L, 4, 128, 2048),
                           w1_t, w2_t], axis=1)  # [L, 48, 128, 2048] bf16
    bqk_t = np.stack([bq_f.reshape(L, ET, 128).transpose(0, 2, 1),
                      bk_f.reshape(L, ET, 128).transpose(0, 2, 1)], axis=1)  # [L,2,128,8]
    br_t = bf(np.stack([bv_f, f["bo"], f["b2"]], axis=1))  # [L,3,E]
    b1_t = np.ascontiguousarray(b1_f.reshape(L, FFT, 128).transpose(0, 2, 1))  # [L,128,32]

    in_maps = []
    for c in range(N_CORES):
        # lm head vocab shard
        sl = slice(c * VS, (c + 1) * VS)
        lw = lmw_f[:, sl]                                  # [E, 4000]
        a = lw.reshape(2, 4, 128, NTV, 500)                # eh el p nt c
        a = a.transpose(3, 0, 2, 1, 4)                     # nt eh p el c
        lmw_t = bf(a.reshape(NTV, 2, 128, 2000))
        lmb_t = bf(lmb_f[sl][None, :])

        # tokens: half0 = seq0 block c, half1 = seq1 block 7-c
        x0_t = np.stack([x_emb[0, c * 128:(c + 1) * 128],
                         x_emb[1, (7 - c) * 128:(8 - c) * 128]]).astype(NPBF16)

        # masks [8, 2, 128, 128] (kt, qt)
        m = np.zeros((N_CORES, 2, 128, 128), np.float32)
        for r in range(N_CORES):
            # half 0: q seq0 block c vs k seq0 block r
            if r < c:
                m[r, 0] = 1.0
            elif r == c:
                m[r, 0] = (np.arange(128)[:, None] <= np.arange(128)[None, :])
            # half 1: q seq1 block 7-c vs k seq1 block 7-r
            if r > c:
                m[r, 1] = 1.0
            elif r == c:
                m[r, 1] = (np.arange(128)[:, None] <= np.arange(128)[None, :])
        in_maps.append({
            "x0": x0_t, "ws": np.ascontiguousarray(allw[:, 6 * c:6 * (c + 1)]),
            "b_qk": bqk_t, "b_r": br_t, "b_1": b1_t,
            "mask": bf(m), "lmw": lmw_t, "lmb": lmb_t,
        })
    return in_maps


def _assemble(results):
    """Per-core logits [2048(ag order), 4000] -> full [2, 1024, 32000] f32."""
    gro = np.empty(2048, np.int64)
    for tb in range(16):
        r, hf = tb // 2, tb % 2
        if hf == 0:
            rows = np.arange(r * 128, (r + 1) * 128)
        else:
            rows = 1024 + np.arange((7 - r) * 128, (8 - r) * 128)
        gro[tb * 128:(tb + 1) * 128] = rows
    out = np.empty((2048, V), np.float32)
    for c in range(N_CORES):
        deq = (results[c]["logits"].reshape(2048, NTV, NTC).astype(np.float32)
               * np.asarray(results[c]["sinv"], np.float32)[:, :, None])
        out[gro, c * VS:(c + 1) * VS] = deq.reshape(2048, VS)
    return out.reshape(B, T, V)


def get_module():
    global _COMPILED
    if _COMPILED is None:
        _COMPILED = build_module()
    return _COMPILED


def kernel(**inputs):
    nc = get_module()
    in_maps = _fold_inputs(inputs)
    res = bass_utils.run_bass_kernel_spmd(nc, in_maps, core_ids=list(range(N_CORES)))
    return _assemble(res.results)


if __name__ == "__main__":
    import reference
    inputs = reference.setup_inputs()
    out = kernel(**{k: np.asarray(v) for k, v in inputs.items()})
    exp = np.asarray(reference.reference(**inputs))
    err = np.abs(out - exp).max() / np.abs(exp).max()
    print("rel err vs reference:", err)

